# revision 40
# baseline (speedup 1.0000x reference)
"""Trainium2 Bass kernel for IntersectionalVolumeRatio.

out[m,n] = exp(sum_d log(softplus(min(Zm,Ze) - max(zm,ze))) - lmv[m])
with men boxes [M=256, D=64] and candidate boxes [N=20000, D=64],
sharded over 8 NeuronCores along the candidate axis (2500 each).

Math (exp domain, as in the 2-LUT baseline, but pairwise sp-products
halve the second LUT pass — sum(ln sp) = sum over pairs ln(sp*sp)):

  u    = e^diff = min(e^Ze, e^Zm) * min(e^-ze, e^-zm)   [1 fused DVE op]
  sp   = ln(1+u)                                        [ACT pass 1, all elems]
  P    = sp_k * sp_{k+1}  (f16, tt 2x)                  [DVE]
  lnP  = ln(P) -> f16                                   [ACT pass 2, 1/2 elems]
  psum = PE window-matmuls sum 16 pair-rows x 2 blocks per mention
  out  = exp(psum + bias) -> bf16                       [per-partition bias]

Partition layout: p = 16*j + i, j = mention slot (8 per supergroup),
i = local dim. d-group k in 0..3 covers dims 16k..16k+15; candidate
exps are stored j-replicated so mention values ride as per-partition
scalars of the fused build op (custom DVE op MINMIN_MUL_ANT,
registered into concourse's table at import; 1x rate but one pass
instead of ts+ts+tt). The issue order is software-pipelined by one
supergroup (tail stages of group i-1 after builds of group i) so the
in-order DVE queue never head-of-line blocks on ACT. Engine budgets
per core ~ DVE 426us (128 builds + 64 pair mults), ACT 418us (ln1p
273 + lnP 139 + epilogue), PE ~240us, all overlapped. GPSIMD is left
idle: on this part it shares its SBUF port with the DVE, so offloading
elementwise work there is ~additive in wall time (measured), despite
what the occupancy model claims.

Host wrapper: module + jitted shard_map executable built once and
cached; device arrays cached and re-uploaded only on content change;
mention log-volumes folded into the epilogue bias. Output returns bf16
(upcast to f32 on host).

_build(reps=R) unrolls the compute loop R times inside the NEFF
(outputs unchanged); timing two dispatch variants (17 vs 33) and
taking the slope isolates on-device execution time. Used by test.py.
"""

import numpy as np

M = 256
D = 64
N = 20000
NCORES = 8
NS = N // NCORES          # 2500 candidates per core
CH = 500                  # PSUM chunk (bank limit is 512 f32)
NCH = NS // CH
NK = 4                    # d-groups (16 dims each)
NSG = 16                  # supergroups per mention half (8 mentions each)

# Tile-pool buffer depths: (ubig, pair, quad, lnq, work)
BUFS = (3, 2, 2, 2, 2)
# Supergroups of issue-distance between builds and their tail stages
# (ubig bufs must be >= PIPE_DEPTH + 1)
PIPE_DEPTH = 2
# Quad products on GPSIMD (False = DVE, bf16 out at 2x). Only used when
# LEVELS == 2. NOTE: on this part GPSIMD shares its SBUF port with the
# DVE, so concurrent GPSIMD work is ~additive with DVE wall time.
QUAD_ON_GP = False
# Reduction levels before the ACT Ln pass: 2 = quads (ln on 1/4 of the
# elements), 1 = pairs only (ln on 1/2; no quad stage at all).
LEVELS = 1
# Supergroup indices (0..31) that skip the pair multiply and take the
# plain 2-LUT path (Ln over all 4 slots, 4 PE a-blocks): trades ACT
# slack for DVE relief to balance the two engines.
NOPAIR = frozenset()

_cache = {}


def _register_custom_op():
    """Register MINMIN_MUL_ANT: out = min(in0, s0) * min(in1, s1).
    Appended to concourse's custom-DVE op table at runtime (row 17 is
    free); the per-NEFF uop table is generated from this spec by the
    normal compile path."""
    import concourse.dve_ops as dve_ops
    if "MINMIN_MUL_ANT" in dve_ops._SUB_OPCODE_FOR_NAME:
        return dve_ops.CUSTOM_DVE_SPECS and next(
            op for op in dve_ops.OPS if op.name == "MINMIN_MUL_ANT")
    from concourse.dve_spec import Spec, Src0, Src1, C0, C1, minn, lower
    from concourse.dve_uop import DveOpSpec

    body = minn(Src0, C0) * minn(Src1, C1)

    def ref(in0, in1, s0, s1, imm2):
        return (np.minimum(in0.astype(np.float32), s0)
                * np.minimum(in1.astype(np.float32), s1))

    row = max(dve_ops._SUB_OPCODE_FOR_NAME.values()) + 1
    assert row < 0x20
    shas = {}
    for ver in ("v3", "v4"):
        uops = lower(Spec(body=body, reference=ref), ver=ver)
        shas[ver] = DveOpSpec(name="MINMIN_MUL_ANT", opcode=row, uops=uops,
                              rd1_en=True).sha(ver)
    op = dve_ops.DveOp("MINMIN_MUL_ANT", Spec(body=body, reference=ref),
                       subdim=False, uops_sha=shas)
    dve_ops.OPS.append(op)
    dve_ops.CUSTOM_DVE_SPECS[op.name] = op.spec
    dve_ops._SUB_OPCODE_FOR_NAME[op.name] = row
    return op


def _build(reps=1, do_compile=True):
    from concourse import bacc, mybir
    from concourse.tile import TileContext

    minmin = _register_custom_op()

    F32 = mybir.dt.float32
    F16 = mybir.dt.float16
    BF16 = mybir.dt.bfloat16
    AF = mybir.ActivationFunctionType
    OP = mybir.AluOpType

    nc = bacc.Bacc("TRN2", target_bir_lowering=False, debug=False,
                   num_devices=NCORES)
    # candidate exps, j-replicated: [p=16j+i, k*NS+n]
    eze = nc.dram_tensor("eze", [128, NK * NS], F32, kind="ExternalInput").ap()
    enze = nc.dram_tensor("enze", [128, NK * NS], F32,
                          kind="ExternalInput").ap()
    # mention exp scalars: col = g*64 + s*4 + k
    ezm = nc.dram_tensor("ezm", [128, 128], F32, kind="ExternalInput").ap()
    enzm = nc.dram_tensor("enzm", [128, 128], F32, kind="ExternalInput").ap()
    nlmv = nc.dram_tensor("nlmv", [128, 2], F32, kind="ExternalInput").ap()
    # 16 PE window matrices, concatenated: wmat[:, 128s:128s+128]
    wmat = nc.dram_tensor("wmat", [128, NSG * 128], F16,
                          kind="ExternalInput").ap()
    out = nc.dram_tensor("out", [M, NS], BF16, kind="ExternalOutput").ap()

    b_u, b_p, b_q, b_l, b_w = BUFS
    with TileContext(nc) as tc:
        with tc.tile_pool(name="persist", bufs=1) as pp, \
             tc.tile_pool(name="ubig", bufs=b_u) as up, \
             tc.tile_pool(name="pair", bufs=b_p) as prp, \
             tc.tile_pool(name="quad", bufs=b_q) as qdp, \
             tc.tile_pool(name="lnq", bufs=b_l) as lqp, \
             tc.tile_pool(name="work", bufs=b_w) as wp, \
             tc.tile_pool(name="psum", bufs=1, space="PSUM") as qp:

            # ---- stage inputs ----
            eze_sb = pp.tile([128, NK * NS], F32, tag="eze")
            enze_sb = pp.tile([128, NK * NS], F32, tag="enze")
            ezm_sb = pp.tile([128, 128], F32, tag="ezm")
            enzm_sb = pp.tile([128, 128], F32, tag="enzm")
            nlmv_sb = pp.tile([128, 2], F32, tag="nlmv")
            # PE windows: Ws[s][p, 8s+j] = 1 iff p//16 == j (host-built;
            # psum accumulates over the 16 sgroup matmuls, each landing
            # its 8 mention rows at output partitions 8s..8s+8)
            wmat_sb = pp.tile([128, NSG * 128], F16, tag="wmat")
            for t_, s_ in [(eze_sb, eze), (enze_sb, enze), (ezm_sb, ezm),
                           (enzm_sb, enzm), (nlmv_sb, nlmv),
                           (wmat_sb, wmat)]:
                nc.sync.dma_start(out=t_[:], in_=s_[:])
            Ws = [wmat_sb[:, 128 * s:128 * (s + 1)] for s in range(NSG)]

            def emit_build(g, s):
                """Build u and issue ln1p for supergroup (g, s).

                Slot order (k0,k2,k1,k3) makes the downstream pair
                multiply a single fully-contiguous tensor_tensor
                (slots 0-1 times slots 2-3 = k0*k1 | k2*k3), which
                keeps the silicon in 2x packed mode."""
                col0 = g * 64 + s * 4
                uT = up.tile([128, NK * NS], F16, tag="uT")
                for slot, k in enumerate((0, 2, 1, 3)):
                    dsl = slice(slot * NS, (slot + 1) * NS)
                    ssl = slice(k * NS, (k + 1) * NS)
                    cs = slice(col0 + k, col0 + k + 1)
                    nc.vector._custom_dve(
                        minmin, out=uT[:, dsl],
                        in0=eze_sb[:, ssl], in1=enze_sb[:, ssl],
                        s0=ezm_sb[:, cs], s1=enzm_sb[:, cs])
                nc.scalar.activation(uT[:], uT[:], AF.Ln, bias=1.0)
                return uT

            def emit_tail(g, s, uT, psums, sgi):
                """Pair/quad/ln/PE stages for an already-built supergroup."""
                if sgi in NOPAIR:
                    # 2-LUT path: lnsp of all 4 slots in place, no DVE
                    # pair work
                    nc.scalar.activation(uT[:], uT[:], AF.Ln)
                    for c in range(NCH):
                        for a in range(NK):
                            nc.tensor.matmul(
                                psums[c][:], lhsT=Ws[s],
                                rhs=uT[:, a * NS + c * CH:
                                       a * NS + (c + 1) * CH],
                                start=(s == 0 and a == 0),
                                stop=(s == NSG - 1 and a == NK - 1))
                    return
                P = prp.tile([128, 2 * NS], F16, tag="P")
                # one fully-contiguous tensor_tensor (2x packed mode):
                # uT slots are ordered (k0,k2,k1,k3) by emit_build, so
                # slots 0-1 times slots 2-3 yields k0*k1 | k2*k3.
                nc.vector.tensor_tensor(
                    P[:], uT[:, 0:2 * NS], uT[:, 2 * NS:4 * NS], OP.mult)
                if LEVELS == 2:
                    Q = qdp.tile([128, NS], F32 if QUAD_ON_GP else BF16,
                                 tag="Q")
                    qeng = nc.gpsimd if QUAD_ON_GP else nc.vector
                    qeng.tensor_tensor(
                        Q[:], P[:, 0:NS], P[:, NS:2 * NS], OP.mult)
                    lnQ = lqp.tile([128, NS], F16, tag="lnQ")
                    nc.scalar.activation(lnQ[:], Q[:], AF.Ln)
                    for c in range(NCH):
                        nc.tensor.matmul(
                            psums[c][:], lhsT=Ws[s],
                            rhs=lnQ[:, c * CH:(c + 1) * CH],
                            start=(s == 0), stop=(s == NSG - 1))
                else:
                    lnP = lqp.tile([128, 2 * NS], F16, tag="lnP")
                    nc.scalar.activation(lnP[:], P[:], AF.Ln)
                    for c in range(NCH):
                        for a in range(2):
                            nc.tensor.matmul(
                                psums[c][:], lhsT=Ws[s],
                                rhs=lnP[:, a * NS + c * CH:
                                        a * NS + (c + 1) * CH],
                                start=(s == 0 and a == 0),
                                stop=(s == NSG - 1 and a == 1))

            def emit_epilogue(g, psums):
                for c in range(NCH):
                    osb = wp.tile([128, CH], BF16, tag="osb")
                    nc.scalar.activation(osb[:], psums[c][:], AF.Exp,
                                         bias=nlmv_sb[:, g:g + 1])
                    nc.sync.dma_start(
                        out=out[g * 128:(g + 1) * 128,
                                c * CH:(c + 1) * CH],
                        in_=osb[:])

            # Software-pipelined by PIPE_DEPTH supergroups: the post-ACT
            # stages of supergroup i-PIPE_DEPTH are issued after the builds
            # of supergroup i so the in-order DVE queue never head-of-line
            # blocks on ACT, and cross-engine semaphore round-trips have
            # extra issue-distance to hide in. The pipeline is carried
            # across reps (drained only once at the very end); PSUM tiles
            # are allocated in the flush path so tag aliasing stays in
            # program order with the matmuls that write them.
            cur_psums = [None]       # psums of the g being flushed

            def flush_one(pend):
                pg, ps, puT = pend.pop(0)
                if ps == 0:
                    cur_psums[0] = [qp.tile([128, CH], F32, name=f"ps{c}",
                                            tag=f"ps{c}")
                                    for c in range(NCH)]
                emit_tail(pg, ps, puT, cur_psums[0], pg * NSG + ps)
                if ps == NSG - 1:
                    emit_epilogue(pg, cur_psums[0])

            pend = []                # [(g, s, uT), ...]
            for _rep in range(reps):
                for sgi in range(2 * NSG):
                    g, s = divmod(sgi, NSG)
                    uT = emit_build(g, s)
                    pend.append((g, s, uT))
                    if len(pend) > PIPE_DEPTH:
                        flush_one(pend)
            while pend:
                flush_one(pend)
    if do_compile:
        nc.compile()
    return nc


def _prep_host(men_embeds, all_en_embeds):
    men = np.ascontiguousarray(np.asarray(men_embeds, dtype=np.float32))
    en = np.ascontiguousarray(np.asarray(all_en_embeds, dtype=np.float32))
    zm, Zm = men[:, :D], men[:, D:]
    # mention scalars: EZM[p=16j+i, col=g*64+s*4+k] = exp(Zm[g*128+8s+j, 16k+i])
    ezm = np.empty((128, 128), dtype=np.float32)
    enzm = np.empty((128, 128), dtype=np.float32)
    zmr = Zm.reshape(2, 16, 8, 4, 16)    # [g, s, j, k, i]
    nzmr = (-zm).reshape(2, 16, 8, 4, 16)
    # [p=j*16+i, col=g*64+s*4+k]
    ezm[:] = np.exp(zmr).transpose(2, 4, 0, 1, 3).reshape(128, 128)
    enzm[:] = np.exp(nzmr).transpose(2, 4, 0, 1, 3).reshape(128, 128)
    # epilogue bias: psum row p of group g is mention m = g*128 + p
    lmv = np.sum(np.log(np.logaddexp(0.0, Zm - zm)), axis=1)  # [M]
    nlmv = np.ascontiguousarray((-lmv).reshape(2, 128).T.astype(np.float32))
    # candidate exps, j-replicated: [16j+i, k*NS+n] = exp(Ze[n,16k+i])
    eze_all = np.empty((NCORES * 128, NK * NS), dtype=np.float32)
    enze_all = np.empty((NCORES * 128, NK * NS), dtype=np.float32)
    for sh in range(NCORES):
        ens = en[sh * NS:(sh + 1) * NS]
        zeT = np.exp(ens[:, D:].T)       # [64, NS] e^Ze
        nzeT = np.exp(-ens[:, :D].T)     # [64, NS] e^-ze
        for k in range(NK):
            blk = slice(k * NS, (k + 1) * NS)
            eze_all[sh * 128:(sh + 1) * 128, blk] = np.tile(
                zeT[16 * k:16 * (k + 1)], (8, 1))
            enze_all[sh * 128:(sh + 1) * 128, blk] = np.tile(
                nzeT[16 * k:16 * (k + 1)], (8, 1))
    # PE window matrices: wmat[p, 128s + 8s+j] = 1 iff p//16 == j
    wmat = np.zeros((128, NSG * 128), dtype=np.float16)
    for s in range(NSG):
        for j in range(8):
            wmat[16 * j:16 * (j + 1), 128 * s + 8 * s + j] = 1.0
    return {"eze": eze_all, "enze": enze_all,
            "ezm": np.tile(ezm, (NCORES, 1)),
            "enzm": np.tile(enzm, (NCORES, 1)),
            "nlmv": np.tile(nlmv, (NCORES, 1)),
            "wmat": np.tile(wmat, (NCORES, 1))}


def _make_executable(nc):
    """Jit a shard_map over 8 cores around the single bass_exec call."""
    import jax
    from jax.sharding import Mesh, PartitionSpec
    from jax.experimental.shard_map import shard_map
    from concourse import mybir
    from concourse.bass2jax import (
        install_neuronx_cc_hook, _bass_exec_p, partition_id_tensor)

    install_neuronx_cc_hook()
    partition_name = (nc.partition_id_tensor.name
                      if nc.partition_id_tensor else None)
    in_names, out_names, out_avals = [], [], []
    for alloc in nc.m.functions[0].allocations:
        if not isinstance(alloc, mybir.MemoryLocationSet):
            continue
        name = alloc.memorylocations[0].name
        if alloc.kind == "ExternalInput":
            if name != partition_name:
                in_names.append(name)
        elif alloc.kind == "ExternalOutput":
            out_names.append(name)
            out_avals.append(jax.core.ShapedArray(
                tuple(alloc.tensor_shape), mybir.dt.np(alloc.dtype)))
    all_in_names = list(in_names)
    if partition_name is not None:
        all_in_names.append(partition_name)

    def _body(*args):
        operands = list(args)
        if partition_name is not None:
            operands.append(partition_id_tensor())
        return tuple(_bass_exec_p.bind(
            *operands,
            out_avals=tuple(out_avals),
            in_names=tuple(all_in_names),
            out_names=tuple(out_names),
            lowering_input_output_aliases=(),
            sim_require_finite=True,
            sim_require_nnan=True,
            nc=nc,
        ))

    devices = jax.devices()[:NCORES]
    mesh = Mesh(np.asarray(devices), ("core",))
    fn = jax.jit(shard_map(
        _body, mesh=mesh,
        in_specs=(PartitionSpec("core"),) * len(in_names),
        out_specs=(PartitionSpec("core"),) * len(out_names),
        check_rep=False))
    return fn, in_names, mesh


def _get_state():
    if "fn" not in _cache:
        nc = _build()
        fn, in_names, mesh = _make_executable(nc)
        _cache.update(nc=nc, fn=fn, in_names=in_names, mesh=mesh,
                      host_in={}, dev_in={})
    return _cache


def _device_inputs(st, host_in):
    """Sharded device_put per input array, reusing cached device arrays
    for arrays whose content is unchanged since the previous call."""
    import jax
    from jax.sharding import NamedSharding, PartitionSpec
    sh = NamedSharding(st["mesh"], PartitionSpec("core"))
    dev = []
    for k in st["in_names"]:
        cached = st["host_in"].get(k)
        if cached is None or not np.array_equal(cached, host_in[k]):
            st["dev_in"][k] = jax.device_put(host_in[k], sh)
            st["host_in"][k] = host_in[k]
        dev.append(st["dev_in"][k])
    return dev


def kernel(men_embeds, all_en_embeds):
    st = _get_state()
    dev = _device_inputs(st, _prep_host(men_embeds, all_en_embeds))
    out_arrs = st["fn"](*dev)
    glob = np.asarray(out_arrs[0])                  # [8*256, 2500] bf16
    out = np.empty((M, N), dtype=np.float32)
    blocks = glob.astype(np.float32).reshape(NCORES, M, NS)
    for sh in range(NCORES):
        out[:, sh * NS:(sh + 1) * NS] = blocks[sh]
    return out


def hw_exec_time_ns(men_embeds, all_en_embeds, r_lo=17, r_hi=33, ntrials=13):
    """Per-execution on-device time, measured as the wall-time slope
    between NEFFs running the compute loop r_lo vs r_hi times internally
    (identical dispatch/transfer overhead cancels out)."""
    import time
    import jax
    st = _get_state()
    dev = _device_inputs(st, _prep_host(men_embeds, all_en_embeds))
    key = ("fns_timing", r_lo, r_hi)
    if key not in st:
        fns = {}
        for r in (r_lo, r_hi):
            fn_r, _, _ = _make_executable(_build(reps=r))
            fns[r] = fn_r
        st[key] = fns
    fns = st[key]
    for fn in fns.values():                        # warm/compile
        jax.block_until_ready(fn(*dev))
    walls = {r: [] for r in fns}
    for _ in range(ntrials):
        for r in (r_lo, r_hi):                     # adjacent in time so
            t0 = time.time()                       # dispatch conditions pair
            jax.block_until_ready(fns[r](*dev))
            walls[r].append(time.time() - t0)
    diffs = sorted(h - l for l, h in zip(walls[r_lo], walls[r_hi]))
    med_diff = diffs[len(diffs) // 2]
    return med_diff / (r_hi - r_lo) * 1e9, walls
